# revision 88
# baseline (speedup 1.0000x reference)
"""DCN-FPN Trainium2 kernel (nn_DCNFPN).

Sharding: 8 cores = 4 images x 2 row-halves. Each core computes rows
[g0, g0+23] of every 40-row intermediate (g0 = 0 top / 16 bottom), with
shrinking-validity redundancy so no cross-core communication is needed:
the correct-row front shrinks by 1 row per DCN iteration and we carry 4
spare rows; host keeps rows 0..19 (top) / 20..39 (bottom) of the output.

Per DCN call (4 calls: levels 0,1,0,1):
  - offset conv (3x3, 256->48) as 36 bf16 matmuls accumulating in PSUM
  - small math on [64,480] tiles (p = yx*32 + rcb*16 + tap) computes
    bilinear corner row-pair indices + 4 slot weights (mask/validity
    folded in); hot ops are fused custom DVE ops (floor via the +2^23
    round trick, clip+1, eq+1, add-add) registered at import
  - gather indices go through DRAM in a q-major layout so the
    %16-wrap lands as one cheap strided DMA per (corner, z), then
    three log-doubling copies replicate rows 0:16 -> 128
  - per tap (16): dma_gather of top/bot 2-pixel row pairs in FP8
    (elem 512B) from the pixel-major feature table; ACT upcasts to
    bf16 and de-interleaves the 16-bit-granularity channel pairs
    (partition cp holds channels 2cp/2cp+1 -> "hl" slot = parity,
    DCN stationary weights are parity-remapped on the host);
    slot-weight broadcast as one DMA per tap from a tap-major DRAM
    wall copy; 4 DVE mults + 2 corner-adds (pads skipped, FD 1920);
    the pixel-pair sum is folded into PSUM accumulation (8 matmuls
    of FD 960 per tap)
  - f += relu(dc + b) fused on DVE (custom op), bf16 shadow for PE
Final: residual conv + fh (bf16), store [256, 960] fp32.

Accuracy: fp8(e4m3) pixel tables add ~1.0% rel error (vs 2% budget);
kernel-side bf16 paths add ~0.3%.

Sample enumeration: i = tap*1024 + z*512 + c (z = rcb, c < 480 real).
Gather idx layout [i%16, i//16] == [rc%16, tap*64 + z*32 + c//16].
"""
import sys
sys.path.insert(0, "/opt/trn_rl_repo")

from contextlib import ExitStack
import numpy as np
import ml_dtypes

import os
import bass_rust
import concourse.bass as bass
import concourse.bacc as bacc
import concourse.mybir as mybir
import concourse.tile as tile

# ---- custom fused DVE ops -------------------------------------------------
from concourse import dve_ops as DOPS
from concourse.dve_spec import (Spec, Src0, Src1, C0, C1, Zero, One,
                                lower as dve_lower, _has_src1, eq, minn, maxx,
                                relu)
from concourse.dve_uop import DveOpSpec


def _register_op(name, spec, subdim=False):
    for o in DOPS.OPS:
        if o.name == name:
            return o
    shas = {}
    for ver in ("v3", "v4"):
        try:
            u = dve_lower(spec, ver=ver)
            s = DveOpSpec(name=name, opcode=0, uops=u, rd1_en=_has_src1(spec))
            shas[ver] = s.sha(ver)
        except Exception:
            pass
    op = DOPS.DveOp(name, spec, subdim=subdim, uops_sha=shas)
    DOPS.OPS.append(op)
    DOPS.CUSTOM_DVE_SPECS[name] = spec
    DOPS._SUB_OPCODE_FOR_NAME[name] = DOPS._CUSTOM_DVE_ROW_BASE + len(DOPS.OPS) - 1
    return op


import numpy as _np


def _rnd_ref(in0, in1, s0, s1, imm2):
    x = _np.asarray(in0, _np.float32)
    m = _np.float32(12582912.0)
    return ((x + m).astype(_np.float32) - m).astype(_np.float32)


OP_ADD_ADD = _register_op("ANT_ADD_ADD_KC", Spec(
    body=(Src0 + Src1) + C0,
    reference=lambda in0, in1, s0, s1, imm2: (in0 + in1) + s0))
# round-to-nearest via the +2^23 trick; HW mislowers the fused
# round+compare+adjust (shared-subexpr cmp), so it is split in two ops
OP_RND = _register_op("ANT_RND_K", Spec(
    body=(Src0 + C0) - C0,
    reference=_rnd_ref))
OP_FLADJ = _register_op("ANT_FLADJ_KC", Spec(
    body=(Src0 - (Src0 > Src1)) - C0,
    reference=lambda in0, in1, s0, s1, imm2: (
        _np.asarray(in0, _np.float32)
        - (_np.asarray(in0, _np.float32) > _np.asarray(in1, _np.float32))
        .astype(_np.float32)) - _np.float32(s0)))
OP_CLIP_P1 = _register_op("ANT_CLIP_P1", Spec(
    body=minn(maxx(Src0 + One, Zero), C0),
    reference=lambda in0, in1, s0, s1, imm2: _np.minimum(
        _np.maximum(in0 + 1.0, 0.0), s0)))
OP_EQ_P1 = _register_op("ANT_EQ_P1", Spec(
    body=eq(Src0 + One, Src1),
    reference=lambda in0, in1, s0, s1, imm2: (in0 + 1.0 == in1).astype(
        _np.float32)))
OP_SUB_SUB = _register_op("ANT_SUB_SUB_KC", Spec(
    body=(Src0 - Src1) - C0,
    reference=lambda in0, in1, s0, s1, imm2: (in0 - in1) - s0))
# f += relu(dc + bias): fuses the ACT relu and the DVE accumulate
def _add_relu_ref(in0, in1, s0, s1, imm2):
    in1 = _np.asarray(in1).reshape(_np.asarray(in0).shape)
    if isinstance(s0, _np.ndarray):
        s0 = s0.reshape((-1,) + (1,) * (in0.ndim - 1))
    return in0 + _np.maximum(in1 + s0, 0)


OP_ADD_RELU = _register_op("ANT_ADD_RELU_B", Spec(
    body=Src0 + relu(Src1 + C0),
    reference=_add_relu_ref))

F32 = mybir.dt.float32
BF16 = mybir.dt.bfloat16
FP8 = mybir.dt.float8e4
I16 = mybir.dt.int16
I32 = mybir.dt.int32
A = mybir.AluOpType
AF = mybir.ActivationFunctionType

B, C, HOUT = 4, 256, 40
CONFIGS = [(4, 2, 1, 1), (4, 4, 3, 3)]   # (k, stride, pad, dil)
HIN = [80, 160]                          # per level l=0 (f1), l=1 (f0)
ROWS = 24                                # out rows per core per call
RC = ROWS * HOUT                         # 960
NT = 16                                  # taps
CALLS = [0, 1, 0, 1]
FW = 42                                  # padded f width
FR = 26                                  # f window rows
FSZ = FR * FW                            # 1092


def ap_with(ap, dims, offset_elems=None):
    v = ap.copy()
    if offset_elems is not None:
        v = v  # offset handled by caller slicing
    v.ap = bass_rust.VecI64Pair(dims)
    return v


def build_program():
    nc = bacc.Bacc("TRN2", target_bir_lowering=False, debug=False)

    dt = {}

    def din(name, shape, dtype=F32):
        dt[name] = nc.dram_tensor(name, shape, dtype, kind="ExternalInput").ap()

    din("fp0", [HIN[1] * HIN[1] + 1, C], FP8)
    din("fp1", [HIN[0] * HIN[0] + 1, C], FP8)
    din("finit", [C, FSZ], F32)
    din("fh", [128, 2 * RC], BF16)
    din("byx", [64, 2 * 480], F32)
    din("hi0", [64, 2], F32)
    din("com_w", [128, 2 * 9 * 2 * 48], BF16)
    din("com_b", [48, 2], F32)
    din("dcn_w", [2, 128, NT * 2 * 2 * 128], BF16)
    din("dcn_b", [128, 4], F32)
    din("res_w", [128, 9 * 2 * 2 * 128], BF16)
    din("res_b", [128, 2], F32)
    out_d = nc.dram_tensor("out", [C, RC], F32, kind="ExternalOutput").ap()

    with tile.TileContext(nc) as tc, ExitStack() as ctx:
        build_body(nc, tc, ctx, dt, out_d)
    nc.compile()
    return nc


def build_body(nc, tc, ctx, dt, out_d):
    cst = ctx.enter_context(tc.tile_pool(name="cst", bufs=1))
    s64p = ctx.enter_context(tc.tile_pool(name="s64p", bufs=10))
    s32p = ctx.enter_context(tc.tile_pool(name="s32p", bufs=6))
    smi = ctx.enter_context(tc.tile_pool(name="smi", bufs=2))
    omp = ctx.enter_context(tc.tile_pool(name="omp", bufs=1))
    gat = ctx.enter_context(tc.tile_pool(name="gat", bufs=2))
    g16 = ctx.enter_context(tc.tile_pool(name="g16", bufs=2))
    wbp = ctx.enter_context(tc.tile_pool(name="wbp", bufs=2))
    pp = ctx.enter_context(tc.tile_pool(name="pp", bufs=3))
    qp = ctx.enter_context(tc.tile_pool(name="qp", bufs=3))
    fup = ctx.enter_context(tc.tile_pool(name="fup", bufs=1))
    pso = ctx.enter_context(tc.tile_pool(name="pso", bufs=1, space="PSUM"))
    psd = ctx.enter_context(tc.tile_pool(name="psd", bufs=1, space="PSUM"))
    drp = ctx.enter_context(tc.tile_pool(name="drp", bufs=2, space="DRAM"))

    # ---- persistent loads (call-1 critical path first: finit -> conv) -----
    fmas, fsh = [], []
    for h in range(2):
        fm = cst.tile([128, FSZ], F32, tag=f"fmas{h}")
        nc.sync.dma_start(fm[:], dt["finit"][128 * h:128 * (h + 1), :])
        fs = cst.tile([128, FSZ], BF16, tag=f"fsh{h}")
        nc.vector.tensor_copy(fs[:], fm[:])
        fmas.append(fm)
        fsh.append(fs)

    com_t = cst.tile([128, 2 * 9 * 2 * 48], BF16, tag="com")
    nc.sync.dma_start(com_t[:], dt["com_w"])
    com_v = com_t[:].rearrange("p (l t i o) -> p l t i o", l=2, t=9, i=2, o=48)

    byx_t = cst.tile([64, 2 * 480], F32, tag="byx")
    nc.sync.dma_start(byx_t[:], dt["byx"])
    hi0_t = cst.tile([64, 2], F32, tag="hi0")
    nc.sync.dma_start(hi0_t[:], dt["hi0"])
    comb_t = cst.tile([48, 2], F32, tag="comb")
    nc.sync.dma_start(comb_t[:], dt["com_b"])
    dcnb_t = cst.tile([128, 4], F32, tag="dcnb")
    nc.sync.dma_start(dcnb_t[:], dt["dcn_b"])
    resb_t = cst.tile([128, 2], F32, tag="resb")
    nc.sync.dma_start(resb_t[:], dt["res_b"])
    fh_t = cst.tile([128, 2 * RC], BF16, tag="fh")
    nc.sync.dma_start(fh_t[:], dt["fh"])

    fp_ap = {0: dt["fp1"], 1: dt["fp0"]}

    # both levels' DCN weights resident (no per-call reload)
    dcn_vs = []
    for lvl in range(2):
        dcn_t = cst.tile([128, NT * 2 * 2 * 128], BF16, tag=f"dcnw{lvl}")
        nc.sync.dma_start(dcn_t[:], dt["dcn_w"][lvl])
        dcn_vs.append(dcn_t[:].rearrange("p (k i o q) -> p k i o q",
                                         k=NT, i=2, o=2, q=128))

    # persistent gather-idx tiles [128, (corner, tap, cc32)], pads zeroed
    # once; ping-pong by call parity so idx prep overlaps previous call
    reps = []
    for rb in range(2):
        r_ = cst.tile([128, 2 * NT * 64], I16, tag=f"rep{rb}")
        nc.vector.memset(r_[:], 0)
        reps.append(r_)

    # ---- per-call ---------------------------------------------------------
    for ci, lvl in enumerate(CALLS):
        k_, st_, pad_, dil_ = CONFIGS[lvl]
        Hin = Win = HIN[lvl]

        dcn_v = dcn_vs[lvl]

        # offset conv: om_ps [48, (z, 512-block)], 480 used per block
        om_ps = pso.tile([48, 1024], F32, tag="omps")
        conv3x3(nc, fsh, lambda ti, ih: com_v[:, lvl, ti, ih], om_ps)

        om01 = omp.tile([32, RC], F32, tag="om01")
        omv0 = om_ps[0:32, :].rearrange("p (z c) -> p z c", z=2)[:, :, 0:480]
        nc.vector.tensor_scalar(om01[:], omv0, comb_t[0:32, lvl:lvl + 1],
                                None, A.add)
        m16 = omp.tile([16, RC], F32, tag="m16")
        omv1 = om_ps[32:48, :].rearrange("p (z c) -> p z c", z=2)[:, :, 0:480]
        nc.scalar.activation(m16[:], omv1, AF.Sigmoid,
                             bias=comb_t[32:48, lvl:lvl + 1])

        # shuffle into [64,480] (p = yx*32 + rcb*16 + t) / [32,480]
        pos0 = s64p.tile([64, 480], F32, tag="s64")
        for yx in range(2):
            for rcb in range(2):
                nc.sync.dma_start(
                    pos0[yx * 32 + rcb * 16: yx * 32 + rcb * 16 + 16, :],
                    om01[yx * 16:yx * 16 + 16, rcb * 480:(rcb + 1) * 480])
        m32 = s32p.tile([32, 480], F32, tag="s32")
        for rcb in range(2):
            nc.sync.dma_start(m32[rcb * 16:rcb * 16 + 16, :],
                              m16[:, rcb * 480:(rcb + 1) * 480])

        # ---- small math ----
        cnt = [0]

        def t64():
            cnt[0] += 1
            return s64p.tile([64, 480], F32, tag="s64", name=f"t64_{ci}_{cnt[0]}")

        def t32():
            cnt[0] += 1
            return s32p.tile([32, 480], F32, tag="s32", name=f"t32_{ci}_{cnt[0]}")

        sh = t64()
        nc.vector._custom_dve(OP_ADD_ADD, out=sh[:], in0=pos0[:],
                              in1=byx_t[:, lvl * 480:(lvl + 1) * 480],
                              s0=1024.0)
        rnd = t64()
        nc.vector._custom_dve(OP_RND, out=rnd[:], in0=sh[:], s0=12582912.0)
        fl = t64()
        nc.vector._custom_dve(OP_FLADJ, out=fl[:], in0=rnd[:], in1=sh[:],
                              s0=1024.0)
        frac = t64()
        nc.vector._custom_dve(OP_SUB_SUB, out=frac[:], in0=sh[:], in1=fl[:],
                              s0=1024.0)
        c0 = t64()
        nc.vector.tensor_scalar(c0[:], fl[:], 0.0, hi0_t[:, lvl:lvl + 1],
                                A.max, A.min)
        cc1 = t64()
        nc.vector._custom_dve(OP_CLIP_P1, out=cc1[0:32, :], in0=fl[0:32, :],
                              s0=float(Hin - 1))
        nc.vector.tensor_copy(cc1[32:64, :], c0[32:64, :])
        # ---- gather indices first (idx chain is the head critical path) ----
        psx = t32()
        nc.vector.tensor_copy(psx[:], c0[32:64, :])
        # i16 [32, (k, q16, cc30)]: value at c=cc*16+q goes to k*480+q*30+cc
        # so the dflat->rep wrap DMA gets contiguous cc-runs
        i16t = smi.tile([32, 960], I16, tag="i16")
        i16w = i16t[:].rearrange("p (k q cc) -> p k cc q", k=2, q=16, cc=30)
        nc.vector.scalar_tensor_tensor(i16w[:, 0], c0[0:32, :], float(Win),
                                       psx[:], A.mult, A.add)
        nc.vector.scalar_tensor_tensor(i16w[:, 1], cc1[0:32, :], float(Win),
                                       psx[:], A.mult, A.add)
        dflat = drp.tile([32, 960], I16, tag="dfl")
        nc.sync.dma_start(dflat[:], i16t[:])
        dfv = dflat[:].rearrange("p c -> (p c)")
        # rep[q, k, t*64+z*32+cc] = dflat[(z*16+t)*960 + k*480 + q*30 + cc]
        rep = reps[ci % 2]
        rep4 = rep[:].rearrange("p (k t cc) -> p k t cc", k=2, t=NT, cc=64)
        for k in range(2):
            for z in range(2):
                src = dfv[z * 16 * 960 + k * 480:
                          z * 16 * 960 + k * 480 + 1].copy()
                src.ap = bass_rust.VecI64Pair([[30, 16], [960, NT], [1, 30]])
                nc.scalar.dma_start(rep4[0:16, k, :, z * 32:z * 32 + 30], src)
        # log-double rows 0:16 -> 128
        for kk in (16, 32, 64):
            nc.scalar.dma_start(rep[kk:2 * kk, :], rep[0:kk, :])

        # ---- bilinear weight chain -----------------------------------------
        V0 = t64()
        nc.vector.tensor_tensor(V0[:], c0[:], fl[:], A.is_equal)
        V1 = t64()
        nc.vector._custom_dve(OP_EQ_P1, out=V1[:], in0=fl[:], in1=cc1[:])
        # x-side ops stay on base-partition 32 (walrus: TT inputs must share
        # base partition); custom DVE ops mislower at base!=0 on HW, so the
        # x-only F0 keeps the stock 2-op form
        psp = t64()
        nc.vector.tensor_scalar(psp[32:64, :], c0[32:64, :], 1.0, None, A.add)
        F0 = t64()
        nc.vector.tensor_tensor(F0[32:64, :], psp[32:64, :], fl[32:64, :],
                                A.is_equal)
        u = t64()
        nc.vector.tensor_scalar(u[:], frac[:], -1.0, 1.0, A.mult, A.add)
        P0 = t64()
        nc.vector.tensor_tensor(P0[:], u[:], V0[:], A.mult)
        P1 = t64()
        nc.vector.tensor_tensor(P1[:], frac[:], V1[:], A.mult)
        xsA = t64()
        nc.vector.tensor_tensor(xsA[32:64, :], P0[32:64, :], P1[32:64, :], A.add)
        xt = t64()
        nc.vector.tensor_tensor(xt[32:64, :], F0[32:64, :], u[32:64, :], A.mult)
        xt2 = t64()
        nc.vector.tensor_tensor(xt2[32:64, :], V0[32:64, :], frac[32:64, :], A.mult)
        xsB = t64()
        nc.vector.tensor_tensor(xsB[32:64, :], xt[32:64, :], xt2[32:64, :], A.add)
        xs0 = t32()
        nc.vector.tensor_copy(xs0[:], xsA[32:64, :])
        xs1 = t32()
        nc.vector.tensor_copy(xs1[:], xsB[32:64, :])
        A0 = t32()
        nc.vector.tensor_tensor(A0[:], P0[0:32, :], m32[:], A.mult)
        A1 = t32()
        nc.vector.tensor_tensor(A1[:], P1[0:32, :], m32[:], A.mult)

        # wall [32, (slot j, c512)] bf16, j = A(top,pix0) B(top,pix1) C D (bot)
        # 512-col blocks: only cols 0:480 are ever read (pad cols unwritten)
        wall = smi.tile([32, 4 * 512], BF16, tag="wall")
        nc.vector.tensor_tensor(wall[:, 0 * 512:0 * 512 + 480], A0[:], xs0[:], A.mult)
        nc.vector.tensor_tensor(wall[:, 1 * 512:1 * 512 + 480], A0[:], xs1[:], A.mult)
        nc.vector.tensor_tensor(wall[:, 2 * 512:2 * 512 + 480], A1[:], xs0[:], A.mult)
        nc.vector.tensor_tensor(wall[:, 3 * 512:3 * 512 + 480], A1[:], xs1[:], A.mult)
        # t-major wall copy in DRAM: addr = t*3840 + z*1920 + j*480 + c
        wdram = drp.tile([16, 3840], BF16, tag="wdram")
        wflat = wdram[:].rearrange("p f -> (p f)")
        for z in range(2):
            wsr = wall[z * 16:z * 16 + 16, :]
            wsr = wsr.rearrange("p (j c) -> p j c", j=4)[:, :, 0:480]
            wdst = wflat[z * 1920:z * 1920 + 1].copy()
            wdst.ap = bass_rust.VecI64Pair([[3840, NT], [480, 4], [1, 480]])
            nc.sync.dma_start(wdst, wsr)



        # dc accumulator [2][128, (z, 512-block)], 480 used per block
        dcs = [psd.tile([128, 1024], F32, tag=f"dc{oh}", name=f"dc_{ci}_{oh}")
               for oh in range(2)]

        fpv = fp_ap[lvl].copy()
        fpv.ap = bass_rust.VecI64Pair([[C, Hin * Win], [1, 2 * C]])

        RCP = 1024
        for t in range(NT):
            # weight broadcast: wb free (z, j, c480), one DMA per tap
            wb = wbp.tile([128, 4 * RC], BF16, tag="wb")
            src = wflat[t * 3840:t * 3840 + 1].copy()
            src.ap = bass_rust.VecI64Pair([[0, 128], [1, 3840]])
            nc.sync.dma_start(wb[:], src)
            wbj = wb[:].rearrange("p (z j c) -> p j z c", z=2, j=4, c=480)

            gts = []
            for corner in range(2):
                # fp8 gather: 16-bit transpose granularity leaves channel
                # PAIRS per partition; partition cp holds ch (2cp, 2cp+1),
                # free = (pix, i, par). ACT upcasts to bf16 and de-interleaves
                # par to the hl slot: g16 free = (pix, par, i).
                g8 = gat.tile([128, 4 * RCP], FP8, tag=f"g8{corner}")
                g8v = g8[:].rearrange("p (j i) -> p j i", j=4)
                nc.gpsimd.dma_gather(
                    g8v, fpv,
                    rep[:, corner * NT * 64 + t * 64:
                        corner * NT * 64 + (t + 1) * 64],
                    RCP, RCP, 2 * C, elem_step=C,
                    transpose=True, single_packet=False)
                g = g16.tile([128, 4 * RCP], BF16, tag=f"g{corner}")
                g8i = g8[:].rearrange("p (pix i par) -> p pix i par",
                                      pix=2, par=2)
                g16o = g[:].rearrange("p (pix par i) -> p pix i par",
                                      pix=2, par=2)
                # split par-wise: each mult consumes one par half, so its
                # gate is half an upcast; POOL (mostly idle) takes one
                # quarter to debottleneck ACT
                nc.scalar.activation(g16o[:, :, :, 0], g8i[:, :, :, 0],
                                     AF.Identity)
                if corner == 0:
                    nc.scalar.activation(g16o[:, :, :, 1], g8i[:, :, :, 1],
                                         AF.Identity)
                else:
                    nc.gpsimd.tensor_copy(g16o[:, :, :, 1], g8i[:, :, :, 1])
                gts.append(g[:].rearrange("p (pix hl z c) -> p hl pix z c",
                                          pix=2, hl=2, z=2))

            ps_ = []
            for corner in range(2):
                for hilo in range(2):
                    p = pp.tile([128, 2 * RC], BF16, tag="p")
                    pv = p[:].rearrange("p (j z c) -> p j z c", j=2, z=2)
                    gsl = gts[corner][:, hilo, :, :, 0:480]     # [128, 2, 2, 480]
                    wsl = wbj[:, 2 * corner:2 * corner + 2]     # [128, 2, 2, 480]
                    nc.vector.tensor_tensor(pv, gsl, wsl, A.mult)
                    ps_.append(p)
            qs = []
            for hilo in range(2):
                q = qp.tile([128, 2 * RC], BF16, tag="q")
                nc.vector.tensor_tensor(q[:], ps_[hilo][:], ps_[2 + hilo][:], A.add)
                qs.append(q)

            for oh in range(2):
                for ih in range(2):
                    for pix in range(2):
                        for z in range(2):
                            nc.tensor.matmul(
                                dcs[oh][:, z * 512:z * 512 + 480],
                                dcn_v[:, t, ih, oh],
                                qs[ih][:, (pix * 2 + z) * 480:
                                        (pix * 2 + z) * 480 + 480],
                                start=(t == 0 and ih == 0 and pix == 0),
                                stop=(t == NT - 1 and ih == 1 and pix == 1))

        # f update: f += relu(dc + b), fused on DVE (per z half: custom DVE
        # ops allow at most 2 free dims)
        for h in range(2):
            fv = fmas[h][:].rearrange("p (r c) -> p r c", c=FW)
            dcv = dcs[h][:].rearrange("p (z c) -> p z c", z=2)[:, :, 0:480]
            dcv = dcv.rearrange("p z (r c) -> p z r c", c=HOUT)
            bias = dcnb_t[:, 2 * lvl + h:2 * lvl + h + 1]
            for z in range(2):
                nc.vector._custom_dve(
                    OP_ADD_RELU,
                    out=fv[:, 1 + z * 12:13 + z * 12, 1:41],
                    in0=fv[:, 1 + z * 12:13 + z * 12, 1:41],
                    in1=dcv[:, z, 0:12, :], s0=bias)
            fsv = fsh[h][:].rearrange("p (r c) -> p r c", c=FW)[:, 1:25, 1:41]
            fiv = fv[:, 1:25, 1:41]
            nc.vector.tensor_copy(fsv, fiv)

    # ---- residual conv + fh ----------------------------------------------
    res_t = cst.tile([128, 9 * 2 * 2 * 128], BF16, tag="dcnw0")
    nc.sync.dma_start(res_t[:], dt["res_w"])
    res_v = res_t[:].rearrange("p (t i o q) -> p t i o q", t=9, i=2, o=2)
    for oh in range(2):
        rps = psd.tile([128, 1024], F32, tag=f"dc{oh}")
        conv3x3(nc, fsh, lambda ti, ih, oh=oh: res_v[:, ti, ih, oh], rps)
        ot = fup.tile([128, RC], F32, tag="ot")
        rpv = rps[:].rearrange("p (z c) -> p z c", z=2)[:, :, 0:480]
        nc.scalar.activation(ot[:], rpv, AF.Identity,
                             bias=resb_t[:, oh:oh + 1])
        nc.vector.tensor_tensor(ot[:], ot[:], fh_t[:, oh * RC:(oh + 1) * RC], A.add)
        nc.sync.dma_start(out_d[128 * oh:128 * (oh + 1), :], ot[:])


def conv3x3(nc, fsh, w_fn, out_ps):
    """3x3 stride-1 conv over the padded f window; out [cout, 960]."""
    taps = [(a, b) for a in (-1, 0, 1) for b in (-1, 0, 1)]
    for ti, (dy, dx) in enumerate(taps):
        for ih in range(2):
            rhs = fsh[ih][:].rearrange("p (r c) -> p r c", c=FW)
            for nh in range(2):
                nc.tensor.matmul(
                    out_ps[:, nh * 512:nh * 512 + 480],
                    w_fn(ti, ih),
                    rhs[:, 1 + dy + nh * 12:1 + dy + nh * 12 + 12,
                        1 + dx:1 + dx + 40],
                    start=(ti == 0 and ih == 0), stop=(ti == 8 and ih == 1))


# ===========================================================================
# host side
# ===========================================================================

def prep_core_inputs(inputs, b, half):
    """Per-core input map for image b, row-half `half` (0=top)."""
    g0 = 0 if half == 0 else 16
    f0 = np.asarray(inputs["f0"][b], np.float32)
    f1 = np.asarray(inputs["f1"][b], np.float32)
    f2 = np.asarray(inputs["f2"][b], np.float32)

    def pix_table(f):
        hw = f.shape[1] * f.shape[2]
        t = np.zeros((hw + 1, C), np.float32)
        t[:hw] = f.transpose(1, 2, 0).reshape(hw, C)
        return t.astype(ml_dtypes.float8_e4m3fn)

    finit = np.zeros((C, FR, FW), np.float32)
    for r in range(FR):
        gr = g0 - 1 + r
        if 0 <= gr < HOUT:
            finit[:, r, 1:41] = f2[:, gr, :]

    # fh as [128, (oh, rc)]
    fh0 = f2[:, g0:g0 + ROWS, :].reshape(C, RC)
    fh = np.concatenate([fh0[:128], fh0[128:]], axis=1)

    byx = np.zeros((2, 64, 480), np.float32)
    hi0 = np.zeros((2, 64, 1), np.float32)
    for lvl in range(2):
        k_, st_, pad_, dil_ = CONFIGS[lvl]
        Hin = HIN[lvl]
        rc = np.arange(480)
        for rcb in range(2):
            rr = (rcb * 480 + rc) // HOUT
            cc = (rcb * 480 + rc) % HOUT
            for t in range(NT):
                byx[lvl, rcb * 16 + t] = st_ * (g0 + rr) - pad_ + (t // k_) * dil_
                byx[lvl, 32 + rcb * 16 + t] = st_ * cc - pad_ + (t % k_) * dil_
        hi0[lvl, 0:32] = Hin - 1
        hi0[lvl, 32:64] = Hin - 2
    byx = byx.transpose(1, 0, 2).reshape(64, 2 * 480)
    hi0 = hi0.transpose(1, 0, 2).reshape(64, 2)

    perm = list(range(0, 32, 2)) + list(range(1, 32, 2)) + list(range(32, 48))
    com_w = np.zeros((2, 9, 2, 128, 48), np.float32)
    com_b = np.zeros((2, 48, 1), np.float32)
    dcn_w = np.zeros((2, NT, 2, 2, 128, 128), np.float32)
    dcn_b = np.zeros((2, 2, 128, 1), np.float32)
    for lvl in range(2):
        cw = np.asarray(inputs[f"com_w{lvl}"], np.float32)[perm]
        cb = np.asarray(inputs[f"com_b{lvl}"], np.float32)[perm]
        for ty in range(3):
            for tx in range(3):
                for ih in range(2):
                    com_w[lvl, ty * 3 + tx, ih] = \
                        cw[:, ih * 128:(ih + 1) * 128, ty, tx].T
        com_b[lvl, :, 0] = cb
        dw = np.asarray(inputs[f"dcn_w{lvl}"], np.float32)
        # fp8 gather leaves ch pairs per partition: row cp of "ih" slot par
        # is input channel 2*cp + par
        for k in range(NT):
            for par in range(2):
                for oh in range(2):
                    dcn_w[lvl, k, par, oh] = dw[oh * 128:(oh + 1) * 128,
                                                par::2,
                                                k // 4, k % 4].T
        db = np.asarray(inputs[f"dcn_b{lvl}"], np.float32)
        dcn_b[lvl, 0, :, 0] = db[:128]
        dcn_b[lvl, 1, :, 0] = db[128:]
    rw = np.asarray(inputs["res_w"], np.float32)
    res_w = np.zeros((9, 2, 2, 128, 128), np.float32)
    for ty in range(3):
        for tx in range(3):
            for ih in range(2):
                for oh in range(2):
                    res_w[ty * 3 + tx, ih, oh] = rw[oh * 128:(oh + 1) * 128,
                                                    ih * 128:(ih + 1) * 128,
                                                    ty, tx].T
    rb = np.asarray(inputs["res_b"], np.float32)
    res_b = np.stack([rb[:128], rb[128:]], axis=1)  # [128, 2]

    # transpose weight stacks to [partition, ...] DRAM layouts
    com_w = com_w.transpose(3, 0, 1, 2, 4).reshape(128, -1)
    com_b = com_b.transpose(1, 0, 2).reshape(48, 2)
    dcn_w = dcn_w.transpose(0, 4, 1, 2, 3, 5).reshape(2, 128, -1)
    dcn_b = dcn_b.transpose(2, 0, 1, 3).reshape(128, 4)
    res_w = res_w.transpose(3, 0, 1, 2, 4).reshape(128, -1)

    return {
        "fp0": pix_table(f0),
        "fp1": pix_table(f1),
        "finit": finit.reshape(C, FSZ),
        "fh": fh.astype(ml_dtypes.bfloat16),
        "byx": byx,
        "hi0": hi0,
        "com_w": com_w.astype(ml_dtypes.bfloat16),
        "com_b": np.ascontiguousarray(com_b),
        "dcn_w": np.ascontiguousarray(dcn_w).astype(ml_dtypes.bfloat16),
        "dcn_b": np.ascontiguousarray(dcn_b),
        "res_w": np.ascontiguousarray(res_w).astype(ml_dtypes.bfloat16),
        "res_b": np.ascontiguousarray(res_b).astype(np.float32),
    }


def assemble_output(results):
    out = np.zeros((B, C, HOUT, HOUT), np.float32)
    for b in range(B):
        top = np.asarray(results[2 * b]["out"]).reshape(C, ROWS, HOUT)
        bot = np.asarray(results[2 * b + 1]["out"]).reshape(C, ROWS, HOUT)
        out[b, :, 0:20, :] = top[:, 0:20, :]
        out[b, :, 20:40, :] = bot[:, 4:24, :]
    return out


_NC_CACHE = []


def kernel(**inputs):
    if not _NC_CACHE:
        _NC_CACHE.append(build_program())
    nc = _NC_CACHE[0]
    in_maps = [prep_core_inputs(inputs, b, half)
               for b in range(B) for half in range(2)]
    from concourse.bass_utils import run_bass_kernel_spmd
    r = run_bass_kernel_spmd(nc, in_maps, list(range(8)))
    return assemble_output(r.results)



# revision 89
# speedup vs baseline: 1.0161x; 1.0161x over previous
"""DCN-FPN Trainium2 kernel (nn_DCNFPN).

Sharding: 8 cores = 4 images x 2 row-halves. Each core computes rows
[g0, g0+23] of every 40-row intermediate (g0 = 0 top / 16 bottom), with
shrinking-validity redundancy so no cross-core communication is needed:
the correct-row front shrinks by 1 row per DCN iteration and we carry 4
spare rows; host keeps rows 0..19 (top) / 20..39 (bottom) of the output.

Per DCN call (4 calls: levels 0,1,0,1):
  - offset conv (3x3, 256->48) as 36 bf16 matmuls accumulating in PSUM
  - small math on [64,480] tiles (p = yx*32 + rcb*16 + tap) computes
    bilinear corner row-pair indices + 4 slot weights (mask/validity
    folded in); hot ops are fused custom DVE ops (floor via the +2^23
    round trick, clip+1, eq+1, add-add) registered at import
  - gather indices go through DRAM in a q-major layout so the
    %16-wrap lands as one cheap strided DMA per (corner, z), then
    three log-doubling copies replicate rows 0:16 -> 128
  - per tap (16): dma_gather of top/bot 2-pixel row pairs in FP8
    (elem 512B) from the pixel-major feature table; ACT upcasts to
    bf16 and de-interleaves the 16-bit-granularity channel pairs
    (partition cp holds channels 2cp/2cp+1 -> "hl" slot = parity,
    DCN stationary weights are parity-remapped on the host);
    slot-weight broadcast as one DMA per tap from a tap-major DRAM
    wall copy; 4 DVE mults + 2 corner-adds (pads skipped, FD 1920);
    the pixel-pair sum is folded into PSUM accumulation (8 matmuls
    of FD 960 per tap)
  - f += relu(dc + b) fused on DVE (custom op), bf16 shadow for PE
Final: residual conv + fh (bf16), store [256, 960] fp32.

Accuracy: fp8(e4m3) pixel tables add ~1.0% rel error (vs 2% budget);
kernel-side bf16 paths add ~0.3%.

Sample enumeration: i = tap*1024 + z*512 + c (z = rcb, c < 480 real).
Gather idx layout [i%16, i//16] == [rc%16, tap*64 + z*32 + c//16].
"""
import sys
sys.path.insert(0, "/opt/trn_rl_repo")

from contextlib import ExitStack
import numpy as np
import ml_dtypes

import os
import bass_rust
import concourse.bass as bass
import concourse.bacc as bacc
import concourse.mybir as mybir
import concourse.tile as tile

# ---- custom fused DVE ops -------------------------------------------------
from concourse import dve_ops as DOPS
from concourse.dve_spec import (Spec, Src0, Src1, C0, C1, Zero, One,
                                lower as dve_lower, _has_src1, eq, minn, maxx,
                                relu)
from concourse.dve_uop import DveOpSpec


def _register_op(name, spec, subdim=False):
    for o in DOPS.OPS:
        if o.name == name:
            return o
    shas = {}
    for ver in ("v3", "v4"):
        try:
            u = dve_lower(spec, ver=ver)
            s = DveOpSpec(name=name, opcode=0, uops=u, rd1_en=_has_src1(spec))
            shas[ver] = s.sha(ver)
        except Exception:
            pass
    op = DOPS.DveOp(name, spec, subdim=subdim, uops_sha=shas)
    DOPS.OPS.append(op)
    DOPS.CUSTOM_DVE_SPECS[name] = spec
    DOPS._SUB_OPCODE_FOR_NAME[name] = DOPS._CUSTOM_DVE_ROW_BASE + len(DOPS.OPS) - 1
    return op


import numpy as _np


def _rnd_ref(in0, in1, s0, s1, imm2):
    x = _np.asarray(in0, _np.float32)
    m = _np.float32(12582912.0)
    return ((x + m).astype(_np.float32) - m).astype(_np.float32)


OP_ADD_ADD = _register_op("ANT_ADD_ADD_KC", Spec(
    body=(Src0 + Src1) + C0,
    reference=lambda in0, in1, s0, s1, imm2: (in0 + in1) + s0))
# round-to-nearest via the +2^23 trick; HW mislowers the fused
# round+compare+adjust (shared-subexpr cmp), so it is split in two ops
OP_RND = _register_op("ANT_RND_K", Spec(
    body=(Src0 + C0) - C0,
    reference=_rnd_ref))
OP_FLADJ = _register_op("ANT_FLADJ_KC", Spec(
    body=(Src0 - (Src0 > Src1)) - C0,
    reference=lambda in0, in1, s0, s1, imm2: (
        _np.asarray(in0, _np.float32)
        - (_np.asarray(in0, _np.float32) > _np.asarray(in1, _np.float32))
        .astype(_np.float32)) - _np.float32(s0)))
OP_CLIP_P1 = _register_op("ANT_CLIP_P1", Spec(
    body=minn(maxx(Src0 + One, Zero), C0),
    reference=lambda in0, in1, s0, s1, imm2: _np.minimum(
        _np.maximum(in0 + 1.0, 0.0), s0)))
OP_EQ_P1 = _register_op("ANT_EQ_P1", Spec(
    body=eq(Src0 + One, Src1),
    reference=lambda in0, in1, s0, s1, imm2: (in0 + 1.0 == in1).astype(
        _np.float32)))
OP_SUB_SUB = _register_op("ANT_SUB_SUB_KC", Spec(
    body=(Src0 - Src1) - C0,
    reference=lambda in0, in1, s0, s1, imm2: (in0 - in1) - s0))
# f += relu(dc + bias): fuses the ACT relu and the DVE accumulate
def _add_relu_ref(in0, in1, s0, s1, imm2):
    in1 = _np.asarray(in1).reshape(_np.asarray(in0).shape)
    if isinstance(s0, _np.ndarray):
        s0 = s0.reshape((-1,) + (1,) * (in0.ndim - 1))
    return in0 + _np.maximum(in1 + s0, 0)


OP_ADD_RELU = _register_op("ANT_ADD_RELU_B", Spec(
    body=Src0 + relu(Src1 + C0),
    reference=_add_relu_ref))

F32 = mybir.dt.float32
BF16 = mybir.dt.bfloat16
FP8 = mybir.dt.float8e4
I16 = mybir.dt.int16
I32 = mybir.dt.int32
A = mybir.AluOpType
AF = mybir.ActivationFunctionType

B, C, HOUT = 4, 256, 40
CONFIGS = [(4, 2, 1, 1), (4, 4, 3, 3)]   # (k, stride, pad, dil)
HIN = [80, 160]                          # per level l=0 (f1), l=1 (f0)
ROWS = 24                                # out rows per core per call
RC = ROWS * HOUT                         # 960
NT = 16                                  # taps
CALLS = [0, 1, 0, 1]
FW = 42                                  # padded f width
FR = 26                                  # f window rows
FSZ = FR * FW                            # 1092


def ap_with(ap, dims, offset_elems=None):
    v = ap.copy()
    if offset_elems is not None:
        v = v  # offset handled by caller slicing
    v.ap = bass_rust.VecI64Pair(dims)
    return v


def build_program():
    nc = bacc.Bacc("TRN2", target_bir_lowering=False, debug=False)

    dt = {}

    def din(name, shape, dtype=F32):
        dt[name] = nc.dram_tensor(name, shape, dtype, kind="ExternalInput").ap()

    din("fp0", [HIN[1] * HIN[1] + 1, C], FP8)
    din("fp1", [HIN[0] * HIN[0] + 1, C], FP8)
    din("finit", [C, FSZ], F32)
    din("fh", [128, 2 * RC], BF16)
    din("byx", [64, 2 * 480], F32)
    din("hi0", [64, 2], F32)
    din("com_w", [128, 2 * 9 * 2 * 48], BF16)
    din("com_b", [48, 2], F32)
    din("dcn_w", [2, 128, NT * 2 * 2 * 128], BF16)
    din("dcn_b", [128, 4], F32)
    din("res_w", [128, 9 * 2 * 2 * 128], BF16)
    din("res_b", [128, 2], F32)
    out_d = nc.dram_tensor("out", [C, RC], F32, kind="ExternalOutput").ap()

    with tile.TileContext(nc) as tc, ExitStack() as ctx:
        build_body(nc, tc, ctx, dt, out_d)
    nc.compile()
    return nc


def build_body(nc, tc, ctx, dt, out_d):
    cst = ctx.enter_context(tc.tile_pool(name="cst", bufs=1))
    s64p = ctx.enter_context(tc.tile_pool(name="s64p", bufs=10))
    s32p = ctx.enter_context(tc.tile_pool(name="s32p", bufs=6))
    smi = ctx.enter_context(tc.tile_pool(name="smi", bufs=2))
    omp = ctx.enter_context(tc.tile_pool(name="omp", bufs=1))
    gat = ctx.enter_context(tc.tile_pool(name="gat", bufs=2))
    g16 = ctx.enter_context(tc.tile_pool(name="g16", bufs=2))
    wbp = ctx.enter_context(tc.tile_pool(name="wbp", bufs=2))
    pp = ctx.enter_context(tc.tile_pool(name="pp", bufs=3))
    qp = ctx.enter_context(tc.tile_pool(name="qp", bufs=3))
    fup = ctx.enter_context(tc.tile_pool(name="fup", bufs=1))
    pso = ctx.enter_context(tc.tile_pool(name="pso", bufs=1, space="PSUM"))
    psd = ctx.enter_context(tc.tile_pool(name="psd", bufs=1, space="PSUM"))
    drp = ctx.enter_context(tc.tile_pool(name="drp", bufs=2, space="DRAM"))

    # ---- persistent loads (call-1 critical path first: finit -> conv) -----
    fmas, fsh = [], []
    for h in range(2):
        fm = cst.tile([128, FSZ], F32, tag=f"fmas{h}")
        nc.sync.dma_start(fm[:], dt["finit"][128 * h:128 * (h + 1), :])
        fs = cst.tile([128, FSZ], BF16, tag=f"fsh{h}")
        nc.vector.tensor_copy(fs[:], fm[:])
        fmas.append(fm)
        fsh.append(fs)

    com_t = cst.tile([128, 2 * 9 * 2 * 48], BF16, tag="com")
    nc.sync.dma_start(com_t[:], dt["com_w"])
    com_v = com_t[:].rearrange("p (l t i o) -> p l t i o", l=2, t=9, i=2, o=48)

    byx_t = cst.tile([64, 2 * 480], F32, tag="byx")
    nc.sync.dma_start(byx_t[:], dt["byx"])
    hi0_t = cst.tile([64, 2], F32, tag="hi0")
    nc.sync.dma_start(hi0_t[:], dt["hi0"])
    comb_t = cst.tile([48, 2], F32, tag="comb")
    nc.sync.dma_start(comb_t[:], dt["com_b"])
    dcnb_t = cst.tile([128, 4], F32, tag="dcnb")
    nc.sync.dma_start(dcnb_t[:], dt["dcn_b"])
    resb_t = cst.tile([128, 2], F32, tag="resb")
    nc.sync.dma_start(resb_t[:], dt["res_b"])
    fh_t = cst.tile([128, 2 * RC], BF16, tag="fh")
    nc.sync.dma_start(fh_t[:], dt["fh"])

    fp_ap = {0: dt["fp1"], 1: dt["fp0"]}

    # both levels' DCN weights resident (no per-call reload)
    dcn_vs = []
    for lvl in range(2):
        dcn_t = cst.tile([128, NT * 2 * 2 * 128], BF16, tag=f"dcnw{lvl}")
        nc.sync.dma_start(dcn_t[:], dt["dcn_w"][lvl])
        dcn_vs.append(dcn_t[:].rearrange("p (k i o q) -> p k i o q",
                                         k=NT, i=2, o=2, q=128))

    # persistent gather-idx tiles [128, (corner, tap, cc32)], pads zeroed
    # once; ping-pong by call parity so idx prep overlaps previous call
    reps = []
    for rb in range(2):
        r_ = cst.tile([128, 2 * NT * 64], I16, tag=f"rep{rb}")
        nc.vector.memset(r_[:], 0)
        reps.append(r_)

    # ---- per-call ---------------------------------------------------------
    for ci, lvl in enumerate(CALLS):
        k_, st_, pad_, dil_ = CONFIGS[lvl]
        Hin = Win = HIN[lvl]

        dcn_v = dcn_vs[lvl]

        # offset conv: om_ps [48, (z, 512-block)], 480 used per block
        om_ps = pso.tile([48, 1024], F32, tag="omps")
        conv3x3(nc, fsh, lambda ti, ih: com_v[:, lvl, ti, ih], om_ps)

        om01 = omp.tile([32, RC], F32, tag="om01")
        omv0 = om_ps[0:32, :].rearrange("p (z c) -> p z c", z=2)[:, :, 0:480]
        nc.vector.tensor_scalar(om01[:], omv0, comb_t[0:32, lvl:lvl + 1],
                                None, A.add)
        m16 = omp.tile([16, RC], F32, tag="m16")
        omv1 = om_ps[32:48, :].rearrange("p (z c) -> p z c", z=2)[:, :, 0:480]
        nc.scalar.activation(m16[:], omv1, AF.Sigmoid,
                             bias=comb_t[32:48, lvl:lvl + 1])

        # shuffle into [64,480] (p = yx*32 + rcb*16 + t) / [32,480]
        pos0 = s64p.tile([64, 480], F32, tag="s64")
        for yx in range(2):
            for rcb in range(2):
                nc.sync.dma_start(
                    pos0[yx * 32 + rcb * 16: yx * 32 + rcb * 16 + 16, :],
                    om01[yx * 16:yx * 16 + 16, rcb * 480:(rcb + 1) * 480])
        m32 = s32p.tile([32, 480], F32, tag="s32")
        for rcb in range(2):
            nc.sync.dma_start(m32[rcb * 16:rcb * 16 + 16, :],
                              m16[:, rcb * 480:(rcb + 1) * 480])

        # ---- small math ----
        cnt = [0]

        def t64():
            cnt[0] += 1
            return s64p.tile([64, 480], F32, tag="s64", name=f"t64_{ci}_{cnt[0]}")

        def t32():
            cnt[0] += 1
            return s32p.tile([32, 480], F32, tag="s32", name=f"t32_{ci}_{cnt[0]}")

        sh = t64()
        nc.vector._custom_dve(OP_ADD_ADD, out=sh[:], in0=pos0[:],
                              in1=byx_t[:, lvl * 480:(lvl + 1) * 480],
                              s0=1024.0)
        rnd = t64()
        nc.vector._custom_dve(OP_RND, out=rnd[:], in0=sh[:], s0=12582912.0)
        fl = t64()
        nc.vector._custom_dve(OP_FLADJ, out=fl[:], in0=rnd[:], in1=sh[:],
                              s0=1024.0)
        frac = t64()
        nc.vector._custom_dve(OP_SUB_SUB, out=frac[:], in0=sh[:], in1=fl[:],
                              s0=1024.0)
        c0 = t64()
        nc.vector.tensor_scalar(c0[:], fl[:], 0.0, hi0_t[:, lvl:lvl + 1],
                                A.max, A.min)
        cc1 = t64()
        nc.vector._custom_dve(OP_CLIP_P1, out=cc1[0:32, :], in0=fl[0:32, :],
                              s0=float(Hin - 1))
        nc.vector.tensor_copy(cc1[32:64, :], c0[32:64, :])
        # ---- gather indices first (idx chain is the head critical path) ----
        psx = t32()
        nc.vector.tensor_copy(psx[:], c0[32:64, :])
        # i16 [32, (k, q16, cc30)]: value at c=cc*16+q goes to k*480+q*30+cc
        # so the dflat->rep wrap DMA gets contiguous cc-runs
        i16t = smi.tile([32, 960], I16, tag="i16")
        i16w = i16t[:].rearrange("p (k q cc) -> p k cc q", k=2, q=16, cc=30)
        nc.vector.scalar_tensor_tensor(i16w[:, 0], c0[0:32, :], float(Win),
                                       psx[:], A.mult, A.add)
        nc.vector.scalar_tensor_tensor(i16w[:, 1], cc1[0:32, :], float(Win),
                                       psx[:], A.mult, A.add)
        dflat = drp.tile([32, 960], I16, tag="dfl")
        nc.sync.dma_start(dflat[:], i16t[:])
        dfv = dflat[:].rearrange("p c -> (p c)")
        # rep[q, k, t*64+z*32+cc] = dflat[(z*16+t)*960 + k*480 + q*30 + cc]
        rep = reps[ci % 2]
        rep4 = rep[:].rearrange("p (k t cc) -> p k t cc", k=2, t=NT, cc=64)
        for k in range(2):
            for z in range(2):
                src = dfv[z * 16 * 960 + k * 480:
                          z * 16 * 960 + k * 480 + 1].copy()
                src.ap = bass_rust.VecI64Pair([[30, 16], [960, NT], [1, 30]])
                nc.scalar.dma_start(rep4[0:16, k, :, z * 32:z * 32 + 30], src)
        # log-double rows 0:16 -> 128
        for kk in (16, 32, 64):
            nc.scalar.dma_start(rep[kk:2 * kk, :], rep[0:kk, :])

        # ---- bilinear weight chain -----------------------------------------
        V0 = t64()
        nc.vector.tensor_tensor(V0[:], c0[:], fl[:], A.is_equal)
        V1 = t64()
        nc.vector._custom_dve(OP_EQ_P1, out=V1[:], in0=fl[:], in1=cc1[:])
        # x-side ops stay on base-partition 32 (walrus: TT inputs must share
        # base partition); custom DVE ops mislower at base!=0 on HW, so the
        # x-only F0 keeps the stock 2-op form
        psp = t64()
        nc.vector.tensor_scalar(psp[32:64, :], c0[32:64, :], 1.0, None, A.add)
        F0 = t64()
        nc.vector.tensor_tensor(F0[32:64, :], psp[32:64, :], fl[32:64, :],
                                A.is_equal)
        u = t64()
        nc.vector.tensor_scalar(u[:], frac[:], -1.0, 1.0, A.mult, A.add)
        P0 = t64()
        nc.vector.tensor_tensor(P0[:], u[:], V0[:], A.mult)
        P1 = t64()
        nc.vector.tensor_tensor(P1[:], frac[:], V1[:], A.mult)
        xsA = t64()
        nc.vector.tensor_tensor(xsA[32:64, :], P0[32:64, :], P1[32:64, :], A.add)
        xt = t64()
        nc.vector.tensor_tensor(xt[32:64, :], F0[32:64, :], u[32:64, :], A.mult)
        xt2 = t64()
        nc.vector.tensor_tensor(xt2[32:64, :], V0[32:64, :], frac[32:64, :], A.mult)
        xsB = t64()
        nc.vector.tensor_tensor(xsB[32:64, :], xt[32:64, :], xt2[32:64, :], A.add)
        xs0 = t32()
        nc.vector.tensor_copy(xs0[:], xsA[32:64, :])
        xs1 = t32()
        nc.vector.tensor_copy(xs1[:], xsB[32:64, :])
        A0 = t32()
        nc.vector.tensor_tensor(A0[:], P0[0:32, :], m32[:], A.mult)
        A1 = t32()
        nc.vector.tensor_tensor(A1[:], P1[0:32, :], m32[:], A.mult)

        # wall [32, (slot j, c512)] bf16, j = A(top,pix0) B(top,pix1) C D (bot)
        # 512-col blocks: only cols 0:480 are ever read (pad cols unwritten)
        wall = smi.tile([32, 4 * 512], BF16, tag="wall")
        nc.vector.tensor_tensor(wall[:, 0 * 512:0 * 512 + 480], A0[:], xs0[:], A.mult)
        nc.vector.tensor_tensor(wall[:, 1 * 512:1 * 512 + 480], A0[:], xs1[:], A.mult)
        nc.vector.tensor_tensor(wall[:, 2 * 512:2 * 512 + 480], A1[:], xs0[:], A.mult)
        nc.vector.tensor_tensor(wall[:, 3 * 512:3 * 512 + 480], A1[:], xs1[:], A.mult)
        # t-major wall copy in DRAM: addr = t*3840 + z*1920 + j*480 + c
        wdram = drp.tile([16, 3840], BF16, tag="wdram")
        wflat = wdram[:].rearrange("p f -> (p f)")
        for z in range(2):
            wsr = wall[z * 16:z * 16 + 16, :]
            wsr = wsr.rearrange("p (j c) -> p j c", j=4)[:, :, 0:480]
            wdst = wflat[z * 1920:z * 1920 + 1].copy()
            wdst.ap = bass_rust.VecI64Pair([[3840, NT], [480, 4], [1, 480]])
            nc.sync.dma_start(wdst, wsr)



        # dc accumulator [2][128, (z, 512-block)], 480 used per block
        dcs = [psd.tile([128, 1024], F32, tag=f"dc{oh}", name=f"dc_{ci}_{oh}")
               for oh in range(2)]

        fpv = fp_ap[lvl].copy()
        fpv.ap = bass_rust.VecI64Pair([[C, Hin * Win], [1, 2 * C]])

        RCP = 1024
        for t in range(NT):
            # weight broadcast: wb free (z, j, c480), one DMA per tap
            wb = wbp.tile([128, 4 * RC], BF16, tag="wb")
            src = wflat[t * 3840:t * 3840 + 1].copy()
            src.ap = bass_rust.VecI64Pair([[0, 128], [1, 3840]])
            nc.sync.dma_start(wb[:], src)
            wbj = wb[:].rearrange("p (z j c) -> p j z c", z=2, j=4, c=480)

            gts = []
            for corner in range(2):
                # fp8 gather: 16-bit transpose granularity leaves channel
                # PAIRS per partition; partition cp holds ch (2cp, 2cp+1),
                # free = (pix, i, par). ACT upcasts to bf16 and de-interleaves
                # par to the hl slot: g16 free = (pix, par, i).
                g8 = gat.tile([128, 4 * RCP], FP8, tag=f"g8{corner}")
                g8v = g8[:].rearrange("p (j i) -> p j i", j=4)
                nc.gpsimd.dma_gather(
                    g8v, fpv,
                    rep[:, corner * NT * 64 + t * 64:
                        corner * NT * 64 + (t + 1) * 64],
                    RCP, RCP, 2 * C, elem_step=C,
                    transpose=True, single_packet=False)
                g = g16.tile([128, 4 * RCP], BF16, tag=f"g{corner}")
                g8i = g8[:].rearrange("p (pix i par) -> p pix i par",
                                      pix=2, par=2)
                g16o = g[:].rearrange("p (pix par i) -> p pix i par",
                                      pix=2, par=2)
                # split par-wise: each mult consumes one par half, so its
                # gate is half an upcast; POOL (mostly idle) takes one
                # quarter to debottleneck ACT. Pad samples (c 480:512 per
                # z-block) are skipped — the mults never read them.
                g8z = g8[:].rearrange("p (pix z c par) -> p par pix z c",
                                      pix=2, z=2, c=512)[:, :, :, :, 0:480]
                g16z = g[:].rearrange("p (pix par z c) -> p par pix z c",
                                      pix=2, par=2, z=2)[:, :, :, :, 0:480]
                nc.scalar.activation(g16z[:, 0], g8z[:, 0], AF.Identity)
                if corner == 0:
                    nc.scalar.activation(g16z[:, 1], g8z[:, 1], AF.Identity)
                else:
                    nc.gpsimd.tensor_copy(g16z[:, 1], g8z[:, 1])
                gts.append(g[:].rearrange("p (pix hl z c) -> p hl pix z c",
                                          pix=2, hl=2, z=2))

            ps_ = []
            for corner in range(2):
                for hilo in range(2):
                    p = pp.tile([128, 2 * RC], BF16, tag="p")
                    pv = p[:].rearrange("p (j z c) -> p j z c", j=2, z=2)
                    gsl = gts[corner][:, hilo, :, :, 0:480]     # [128, 2, 2, 480]
                    wsl = wbj[:, 2 * corner:2 * corner + 2]     # [128, 2, 2, 480]
                    nc.vector.tensor_tensor(pv, gsl, wsl, A.mult)
                    ps_.append(p)
            qs = []
            for hilo in range(2):
                q = qp.tile([128, 2 * RC], BF16, tag="q")
                nc.vector.tensor_tensor(q[:], ps_[hilo][:], ps_[2 + hilo][:], A.add)
                qs.append(q)

            for oh in range(2):
                for ih in range(2):
                    for pix in range(2):
                        for z in range(2):
                            nc.tensor.matmul(
                                dcs[oh][:, z * 512:z * 512 + 480],
                                dcn_v[:, t, ih, oh],
                                qs[ih][:, (pix * 2 + z) * 480:
                                        (pix * 2 + z) * 480 + 480],
                                start=(t == 0 and ih == 0 and pix == 0),
                                stop=(t == NT - 1 and ih == 1 and pix == 1))

        # f update: f += relu(dc + b), fused on DVE (per z half: custom DVE
        # ops allow at most 2 free dims)
        for h in range(2):
            fv = fmas[h][:].rearrange("p (r c) -> p r c", c=FW)
            dcv = dcs[h][:].rearrange("p (z c) -> p z c", z=2)[:, :, 0:480]
            dcv = dcv.rearrange("p z (r c) -> p z r c", c=HOUT)
            bias = dcnb_t[:, 2 * lvl + h:2 * lvl + h + 1]
            for z in range(2):
                nc.vector._custom_dve(
                    OP_ADD_RELU,
                    out=fv[:, 1 + z * 12:13 + z * 12, 1:41],
                    in0=fv[:, 1 + z * 12:13 + z * 12, 1:41],
                    in1=dcv[:, z, 0:12, :], s0=bias)
            fsv = fsh[h][:].rearrange("p (r c) -> p r c", c=FW)[:, 1:25, 1:41]
            fiv = fv[:, 1:25, 1:41]
            nc.vector.tensor_copy(fsv, fiv)

    # ---- residual conv + fh ----------------------------------------------
    res_t = cst.tile([128, 9 * 2 * 2 * 128], BF16, tag="dcnw0")
    nc.sync.dma_start(res_t[:], dt["res_w"])
    res_v = res_t[:].rearrange("p (t i o q) -> p t i o q", t=9, i=2, o=2)
    for oh in range(2):
        rps = psd.tile([128, 1024], F32, tag=f"dc{oh}")
        conv3x3(nc, fsh, lambda ti, ih, oh=oh: res_v[:, ti, ih, oh], rps)
        ot = fup.tile([128, RC], F32, tag="ot")
        rpv = rps[:].rearrange("p (z c) -> p z c", z=2)[:, :, 0:480]
        nc.scalar.activation(ot[:], rpv, AF.Identity,
                             bias=resb_t[:, oh:oh + 1])
        nc.vector.tensor_tensor(ot[:], ot[:], fh_t[:, oh * RC:(oh + 1) * RC], A.add)
        nc.sync.dma_start(out_d[128 * oh:128 * (oh + 1), :], ot[:])


def conv3x3(nc, fsh, w_fn, out_ps):
    """3x3 stride-1 conv over the padded f window; out [cout, 960]."""
    taps = [(a, b) for a in (-1, 0, 1) for b in (-1, 0, 1)]
    for ti, (dy, dx) in enumerate(taps):
        for ih in range(2):
            rhs = fsh[ih][:].rearrange("p (r c) -> p r c", c=FW)
            for nh in range(2):
                nc.tensor.matmul(
                    out_ps[:, nh * 512:nh * 512 + 480],
                    w_fn(ti, ih),
                    rhs[:, 1 + dy + nh * 12:1 + dy + nh * 12 + 12,
                        1 + dx:1 + dx + 40],
                    start=(ti == 0 and ih == 0), stop=(ti == 8 and ih == 1))


# ===========================================================================
# host side
# ===========================================================================

def prep_core_inputs(inputs, b, half):
    """Per-core input map for image b, row-half `half` (0=top)."""
    g0 = 0 if half == 0 else 16
    f0 = np.asarray(inputs["f0"][b], np.float32)
    f1 = np.asarray(inputs["f1"][b], np.float32)
    f2 = np.asarray(inputs["f2"][b], np.float32)

    def pix_table(f):
        hw = f.shape[1] * f.shape[2]
        t = np.zeros((hw + 1, C), np.float32)
        t[:hw] = f.transpose(1, 2, 0).reshape(hw, C)
        return t.astype(ml_dtypes.float8_e4m3fn)

    finit = np.zeros((C, FR, FW), np.float32)
    for r in range(FR):
        gr = g0 - 1 + r
        if 0 <= gr < HOUT:
            finit[:, r, 1:41] = f2[:, gr, :]

    # fh as [128, (oh, rc)]
    fh0 = f2[:, g0:g0 + ROWS, :].reshape(C, RC)
    fh = np.concatenate([fh0[:128], fh0[128:]], axis=1)

    byx = np.zeros((2, 64, 480), np.float32)
    hi0 = np.zeros((2, 64, 1), np.float32)
    for lvl in range(2):
        k_, st_, pad_, dil_ = CONFIGS[lvl]
        Hin = HIN[lvl]
        rc = np.arange(480)
        for rcb in range(2):
            rr = (rcb * 480 + rc) // HOUT
            cc = (rcb * 480 + rc) % HOUT
            for t in range(NT):
                byx[lvl, rcb * 16 + t] = st_ * (g0 + rr) - pad_ + (t // k_) * dil_
                byx[lvl, 32 + rcb * 16 + t] = st_ * cc - pad_ + (t % k_) * dil_
        hi0[lvl, 0:32] = Hin - 1
        hi0[lvl, 32:64] = Hin - 2
    byx = byx.transpose(1, 0, 2).reshape(64, 2 * 480)
    hi0 = hi0.transpose(1, 0, 2).reshape(64, 2)

    perm = list(range(0, 32, 2)) + list(range(1, 32, 2)) + list(range(32, 48))
    com_w = np.zeros((2, 9, 2, 128, 48), np.float32)
    com_b = np.zeros((2, 48, 1), np.float32)
    dcn_w = np.zeros((2, NT, 2, 2, 128, 128), np.float32)
    dcn_b = np.zeros((2, 2, 128, 1), np.float32)
    for lvl in range(2):
        cw = np.asarray(inputs[f"com_w{lvl}"], np.float32)[perm]
        cb = np.asarray(inputs[f"com_b{lvl}"], np.float32)[perm]
        for ty in range(3):
            for tx in range(3):
                for ih in range(2):
                    com_w[lvl, ty * 3 + tx, ih] = \
                        cw[:, ih * 128:(ih + 1) * 128, ty, tx].T
        com_b[lvl, :, 0] = cb
        dw = np.asarray(inputs[f"dcn_w{lvl}"], np.float32)
        # fp8 gather leaves ch pairs per partition: row cp of "ih" slot par
        # is input channel 2*cp + par
        for k in range(NT):
            for par in range(2):
                for oh in range(2):
                    dcn_w[lvl, k, par, oh] = dw[oh * 128:(oh + 1) * 128,
                                                par::2,
                                                k // 4, k % 4].T
        db = np.asarray(inputs[f"dcn_b{lvl}"], np.float32)
        dcn_b[lvl, 0, :, 0] = db[:128]
        dcn_b[lvl, 1, :, 0] = db[128:]
    rw = np.asarray(inputs["res_w"], np.float32)
    res_w = np.zeros((9, 2, 2, 128, 128), np.float32)
    for ty in range(3):
        for tx in range(3):
            for ih in range(2):
                for oh in range(2):
                    res_w[ty * 3 + tx, ih, oh] = rw[oh * 128:(oh + 1) * 128,
                                                    ih * 128:(ih + 1) * 128,
                                                    ty, tx].T
    rb = np.asarray(inputs["res_b"], np.float32)
    res_b = np.stack([rb[:128], rb[128:]], axis=1)  # [128, 2]

    # transpose weight stacks to [partition, ...] DRAM layouts
    com_w = com_w.transpose(3, 0, 1, 2, 4).reshape(128, -1)
    com_b = com_b.transpose(1, 0, 2).reshape(48, 2)
    dcn_w = dcn_w.transpose(0, 4, 1, 2, 3, 5).reshape(2, 128, -1)
    dcn_b = dcn_b.transpose(2, 0, 1, 3).reshape(128, 4)
    res_w = res_w.transpose(3, 0, 1, 2, 4).reshape(128, -1)

    return {
        "fp0": pix_table(f0),
        "fp1": pix_table(f1),
        "finit": finit.reshape(C, FSZ),
        "fh": fh.astype(ml_dtypes.bfloat16),
        "byx": byx,
        "hi0": hi0,
        "com_w": com_w.astype(ml_dtypes.bfloat16),
        "com_b": np.ascontiguousarray(com_b),
        "dcn_w": np.ascontiguousarray(dcn_w).astype(ml_dtypes.bfloat16),
        "dcn_b": np.ascontiguousarray(dcn_b),
        "res_w": np.ascontiguousarray(res_w).astype(ml_dtypes.bfloat16),
        "res_b": np.ascontiguousarray(res_b).astype(np.float32),
    }


def assemble_output(results):
    out = np.zeros((B, C, HOUT, HOUT), np.float32)
    for b in range(B):
        top = np.asarray(results[2 * b]["out"]).reshape(C, ROWS, HOUT)
        bot = np.asarray(results[2 * b + 1]["out"]).reshape(C, ROWS, HOUT)
        out[b, :, 0:20, :] = top[:, 0:20, :]
        out[b, :, 20:40, :] = bot[:, 4:24, :]
    return out


_NC_CACHE = []


def kernel(**inputs):
    if not _NC_CACHE:
        _NC_CACHE.append(build_program())
    nc = _NC_CACHE[0]
    in_maps = [prep_core_inputs(inputs, b, half)
               for b in range(B) for half in range(2)]
    from concourse.bass_utils import run_bass_kernel_spmd
    r = run_bass_kernel_spmd(nc, in_maps, list(range(8)))
    return assemble_output(r.results)



# revision 90
# speedup vs baseline: 1.0639x; 1.0470x over previous
"""DCN-FPN Trainium2 kernel (nn_DCNFPN).

Sharding: 8 cores = 4 images x 2 row-halves. Each core computes rows
[g0, g0+23] of every 40-row intermediate (g0 = 0 top / 16 bottom), with
shrinking-validity redundancy so no cross-core communication is needed:
the correct-row front shrinks by 1 row per DCN iteration and we carry 4
spare rows; host keeps rows 0..19 (top) / 20..39 (bottom) of the output.

Per DCN call (4 calls: levels 0,1,0,1):
  - offset conv (3x3, 256->48) as 36 bf16 matmuls accumulating in PSUM
  - small math on [64,480] tiles (p = yx*32 + rcb*16 + tap) computes
    bilinear corner row-pair indices + 4 slot weights (mask/validity
    folded in); hot ops are fused custom DVE ops (floor via the +2^23
    round trick, clip+1, eq+1, add-add) registered at import
  - gather indices go through DRAM in a q-major layout so the
    %16-wrap lands as one cheap strided DMA per (corner, z), then
    three log-doubling copies replicate rows 0:16 -> 128
  - per tap (16): dma_gather of top/bot 2-pixel row pairs in FP8
    (elem 512B) from the pixel-major feature table; ACT upcasts to
    bf16 and de-interleaves the 16-bit-granularity channel pairs
    (partition cp holds channels 2cp/2cp+1 -> "hl" slot = parity,
    DCN stationary weights are parity-remapped on the host);
    slot-weight broadcast as one DMA per tap from a tap-major DRAM
    wall copy; 4 DVE mults + 2 corner-adds (pads skipped, FD 1920);
    the pixel-pair sum is folded into PSUM accumulation (8 matmuls
    of FD 960 per tap)
  - f += relu(dc + b) fused on DVE (custom op), bf16 shadow for PE
Final: residual conv + fh (bf16), store [256, 960] fp32.

Accuracy: fp8(e4m3) pixel tables add ~1.0% rel error (vs 2% budget);
kernel-side bf16 paths add ~0.3%.

Sample enumeration: i = tap*1024 + z*512 + c (z = rcb, c < 480 real).
Gather idx layout [i%16, i//16] == [rc%16, tap*64 + z*32 + c//16].
"""
import sys
sys.path.insert(0, "/opt/trn_rl_repo")

from contextlib import ExitStack
import numpy as np
import ml_dtypes

import os
import bass_rust
import concourse.bass as bass
import concourse.bacc as bacc
import concourse.mybir as mybir
import concourse.tile as tile

# ---- custom fused DVE ops -------------------------------------------------
from concourse import dve_ops as DOPS
from concourse.dve_spec import (Spec, Src0, Src1, C0, C1, Zero, One,
                                lower as dve_lower, _has_src1, eq, minn, maxx,
                                relu)
from concourse.dve_uop import DveOpSpec


def _register_op(name, spec, subdim=False):
    for o in DOPS.OPS:
        if o.name == name:
            return o
    shas = {}
    for ver in ("v3", "v4"):
        try:
            u = dve_lower(spec, ver=ver)
            s = DveOpSpec(name=name, opcode=0, uops=u, rd1_en=_has_src1(spec))
            shas[ver] = s.sha(ver)
        except Exception:
            pass
    op = DOPS.DveOp(name, spec, subdim=subdim, uops_sha=shas)
    DOPS.OPS.append(op)
    DOPS.CUSTOM_DVE_SPECS[name] = spec
    DOPS._SUB_OPCODE_FOR_NAME[name] = DOPS._CUSTOM_DVE_ROW_BASE + len(DOPS.OPS) - 1
    return op


import numpy as _np


def _rnd_ref(in0, in1, s0, s1, imm2):
    x = _np.asarray(in0, _np.float32)
    m = _np.float32(12582912.0)
    return ((x + m).astype(_np.float32) - m).astype(_np.float32)


OP_ADD_ADD = _register_op("ANT_ADD_ADD_KC", Spec(
    body=(Src0 + Src1) + C0,
    reference=lambda in0, in1, s0, s1, imm2: (in0 + in1) + s0))
# round-to-nearest via the +2^23 trick; HW mislowers the fused
# round+compare+adjust (shared-subexpr cmp), so it is split in two ops
OP_RND = _register_op("ANT_RND_K", Spec(
    body=(Src0 + C0) - C0,
    reference=_rnd_ref))
OP_FLADJ = _register_op("ANT_FLADJ_KC", Spec(
    body=(Src0 - (Src0 > Src1)) - C0,
    reference=lambda in0, in1, s0, s1, imm2: (
        _np.asarray(in0, _np.float32)
        - (_np.asarray(in0, _np.float32) > _np.asarray(in1, _np.float32))
        .astype(_np.float32)) - _np.float32(s0)))
OP_CLIP_P1 = _register_op("ANT_CLIP_P1", Spec(
    body=minn(maxx(Src0 + One, Zero), C0),
    reference=lambda in0, in1, s0, s1, imm2: _np.minimum(
        _np.maximum(in0 + 1.0, 0.0), s0)))
OP_EQ_P1 = _register_op("ANT_EQ_P1", Spec(
    body=eq(Src0 + One, Src1),
    reference=lambda in0, in1, s0, s1, imm2: (in0 + 1.0 == in1).astype(
        _np.float32)))
OP_SUB_SUB = _register_op("ANT_SUB_SUB_KC", Spec(
    body=(Src0 - Src1) - C0,
    reference=lambda in0, in1, s0, s1, imm2: (in0 - in1) - s0))
# f += relu(dc + bias): fuses the ACT relu and the DVE accumulate
def _add_relu_ref(in0, in1, s0, s1, imm2):
    in1 = _np.asarray(in1).reshape(_np.asarray(in0).shape)
    if isinstance(s0, _np.ndarray):
        s0 = s0.reshape((-1,) + (1,) * (in0.ndim - 1))
    return in0 + _np.maximum(in1 + s0, 0)


OP_ADD_RELU = _register_op("ANT_ADD_RELU_B", Spec(
    body=Src0 + relu(Src1 + C0),
    reference=_add_relu_ref))

F32 = mybir.dt.float32
BF16 = mybir.dt.bfloat16
FP8 = mybir.dt.float8e4
I16 = mybir.dt.int16
I32 = mybir.dt.int32
A = mybir.AluOpType
AF = mybir.ActivationFunctionType

B, C, HOUT = 4, 256, 40
CONFIGS = [(4, 2, 1, 1), (4, 4, 3, 3)]   # (k, stride, pad, dil)
HIN = [80, 160]                          # per level l=0 (f1), l=1 (f0)
ROWS = 24                                # out rows per core per call
RC = ROWS * HOUT                         # 960
NT = 16                                  # taps
CALLS = [0, 1, 0, 1]
FW = 42                                  # padded f width
FR = 26                                  # f window rows
FSZ = FR * FW                            # 1092


def ap_with(ap, dims, offset_elems=None):
    v = ap.copy()
    if offset_elems is not None:
        v = v  # offset handled by caller slicing
    v.ap = bass_rust.VecI64Pair(dims)
    return v


def build_program():
    nc = bacc.Bacc("TRN2", target_bir_lowering=False, debug=False)

    dt = {}

    def din(name, shape, dtype=F32):
        dt[name] = nc.dram_tensor(name, shape, dtype, kind="ExternalInput").ap()

    din("fp0", [HIN[1] * HIN[1] + 1, C], FP8)
    din("fp1", [HIN[0] * HIN[0] + 1, C], FP8)
    din("finit", [C, FSZ], F32)
    din("fh", [128, 2 * RC], BF16)
    din("byx", [64, 2 * 480], F32)
    din("hi0", [64, 2], F32)
    din("com_w", [128, 2 * 9 * 2 * 48], BF16)
    din("com_b", [48, 2], F32)
    din("dcn_w", [2, 128, NT * 2 * 2 * 128], BF16)
    din("dcn_b", [128, 4], F32)
    din("res_w", [128, 9 * 2 * 2 * 128], BF16)
    din("res_b", [128, 2], F32)
    out_d = nc.dram_tensor("out", [C, RC], F32, kind="ExternalOutput").ap()

    with tile.TileContext(nc) as tc, ExitStack() as ctx:
        build_body(nc, tc, ctx, dt, out_d)
    nc.compile()
    return nc


def build_body(nc, tc, ctx, dt, out_d):
    cst = ctx.enter_context(tc.tile_pool(name="cst", bufs=1))
    s64p = ctx.enter_context(tc.tile_pool(name="s64p", bufs=10))
    s32p = ctx.enter_context(tc.tile_pool(name="s32p", bufs=6))
    smi = ctx.enter_context(tc.tile_pool(name="smi", bufs=2))
    omp = ctx.enter_context(tc.tile_pool(name="omp", bufs=1))
    gat = ctx.enter_context(tc.tile_pool(name="gat", bufs=2))
    g16 = ctx.enter_context(tc.tile_pool(name="g16", bufs=2))
    wbp = ctx.enter_context(tc.tile_pool(name="wbp", bufs=2))
    pp = ctx.enter_context(tc.tile_pool(name="pp", bufs=3))
    qp = ctx.enter_context(tc.tile_pool(name="qp", bufs=3))
    fup = ctx.enter_context(tc.tile_pool(name="fup", bufs=1))
    pso = ctx.enter_context(tc.tile_pool(name="pso", bufs=1, space="PSUM"))
    psd = ctx.enter_context(tc.tile_pool(name="psd", bufs=1, space="PSUM"))
    drp = ctx.enter_context(tc.tile_pool(name="drp", bufs=2, space="DRAM"))

    # ---- persistent loads (call-1 critical path first: finit -> conv) -----
    fmas, fsh = [], []
    for h in range(2):
        fm = cst.tile([128, FSZ], F32, tag=f"fmas{h}")
        nc.sync.dma_start(fm[:], dt["finit"][128 * h:128 * (h + 1), :])
        fs = cst.tile([128, FSZ], BF16, tag=f"fsh{h}")
        nc.vector.tensor_copy(fs[:], fm[:])
        fmas.append(fm)
        fsh.append(fs)

    com_t = cst.tile([128, 2 * 9 * 2 * 48], BF16, tag="com")
    nc.sync.dma_start(com_t[:], dt["com_w"])
    com_v = com_t[:].rearrange("p (l t i o) -> p l t i o", l=2, t=9, i=2, o=48)

    byx_t = cst.tile([64, 2 * 480], F32, tag="byx")
    nc.sync.dma_start(byx_t[:], dt["byx"])
    hi0_t = cst.tile([64, 2], F32, tag="hi0")
    nc.sync.dma_start(hi0_t[:], dt["hi0"])
    comb_t = cst.tile([48, 2], F32, tag="comb")
    nc.sync.dma_start(comb_t[:], dt["com_b"])
    dcnb_t = cst.tile([128, 4], F32, tag="dcnb")
    nc.sync.dma_start(dcnb_t[:], dt["dcn_b"])
    resb_t = cst.tile([128, 2], F32, tag="resb")
    nc.sync.dma_start(resb_t[:], dt["res_b"])
    fh_t = cst.tile([128, 2 * RC], BF16, tag="fh")
    nc.sync.dma_start(fh_t[:], dt["fh"])

    fp_ap = {0: dt["fp1"], 1: dt["fp0"]}

    # both levels' DCN weights resident (no per-call reload)
    dcn_vs = []
    for lvl in range(2):
        dcn_t = cst.tile([128, NT * 2 * 2 * 128], BF16, tag=f"dcnw{lvl}")
        nc.sync.dma_start(dcn_t[:], dt["dcn_w"][lvl])
        dcn_vs.append(dcn_t[:].rearrange("p (k i o q) -> p k i o q",
                                         k=NT, i=2, o=2, q=128))

    # persistent gather-idx tiles [128, (corner, tap, cc32)], pads zeroed
    # once; ping-pong by call parity so idx prep overlaps previous call
    reps = []
    for rb in range(2):
        r_ = cst.tile([128, 2 * NT * 64], I16, tag=f"rep{rb}")
        nc.vector.memset(r_[:], 0)
        reps.append(r_)

    # ---- per-call ---------------------------------------------------------
    for ci, lvl in enumerate(CALLS):
        k_, st_, pad_, dil_ = CONFIGS[lvl]
        Hin = Win = HIN[lvl]

        dcn_v = dcn_vs[lvl]

        # offset conv: om_ps [48, (z, 512-block)], 480 used per block
        om_ps = pso.tile([48, 1024], F32, tag="omps")
        conv3x3(nc, fsh, lambda ti, ih: com_v[:, lvl, ti, ih], om_ps)

        om01 = omp.tile([32, RC], F32, tag="om01")
        omv0 = om_ps[0:32, :].rearrange("p (z c) -> p z c", z=2)[:, :, 0:480]
        nc.vector.tensor_scalar(om01[:], omv0, comb_t[0:32, lvl:lvl + 1],
                                None, A.add)
        m16 = omp.tile([16, RC], F32, tag="m16")
        omv1 = om_ps[32:48, :].rearrange("p (z c) -> p z c", z=2)[:, :, 0:480]
        nc.scalar.activation(m16[:], omv1, AF.Sigmoid,
                             bias=comb_t[32:48, lvl:lvl + 1])

        # shuffle into [64,480] (p = yx*32 + rcb*16 + t) / [32,480]
        pos0 = s64p.tile([64, 480], F32, tag="s64")
        for yx in range(2):
            for rcb in range(2):
                nc.sync.dma_start(
                    pos0[yx * 32 + rcb * 16: yx * 32 + rcb * 16 + 16, :],
                    om01[yx * 16:yx * 16 + 16, rcb * 480:(rcb + 1) * 480])
        m32 = s32p.tile([32, 480], F32, tag="s32")
        for rcb in range(2):
            nc.sync.dma_start(m32[rcb * 16:rcb * 16 + 16, :],
                              m16[:, rcb * 480:(rcb + 1) * 480])

        # ---- small math ----
        cnt = [0]

        def t64():
            cnt[0] += 1
            return s64p.tile([64, 480], F32, tag="s64", name=f"t64_{ci}_{cnt[0]}")

        def t32():
            cnt[0] += 1
            return s32p.tile([32, 480], F32, tag="s32", name=f"t32_{ci}_{cnt[0]}")

        sh = t64()
        nc.vector._custom_dve(OP_ADD_ADD, out=sh[:], in0=pos0[:],
                              in1=byx_t[:, lvl * 480:(lvl + 1) * 480],
                              s0=1024.0)
        rnd = t64()
        nc.vector._custom_dve(OP_RND, out=rnd[:], in0=sh[:], s0=12582912.0)
        fl = t64()
        nc.vector._custom_dve(OP_FLADJ, out=fl[:], in0=rnd[:], in1=sh[:],
                              s0=1024.0)
        frac = t64()
        nc.vector._custom_dve(OP_SUB_SUB, out=frac[:], in0=sh[:], in1=fl[:],
                              s0=1024.0)
        c0 = t64()
        nc.vector.tensor_scalar(c0[:], fl[:], 0.0, hi0_t[:, lvl:lvl + 1],
                                A.max, A.min)
        cc1 = t64()
        nc.vector._custom_dve(OP_CLIP_P1, out=cc1[0:32, :], in0=fl[0:32, :],
                              s0=float(Hin - 1))
        nc.vector.tensor_copy(cc1[32:64, :], c0[32:64, :])
        # ---- gather indices first (idx chain is the head critical path) ----
        psx = t32()
        nc.vector.tensor_copy(psx[:], c0[32:64, :])
        # i16 [32, (k, q16, cc30)]: value at c=cc*16+q goes to k*480+q*30+cc
        # so the dflat->rep wrap DMA gets contiguous cc-runs
        i16t = smi.tile([32, 960], I16, tag="i16")
        i16w = i16t[:].rearrange("p (k q cc) -> p k cc q", k=2, q=16, cc=30)
        nc.vector.scalar_tensor_tensor(i16w[:, 0], c0[0:32, :], float(Win),
                                       psx[:], A.mult, A.add)
        nc.vector.scalar_tensor_tensor(i16w[:, 1], cc1[0:32, :], float(Win),
                                       psx[:], A.mult, A.add)
        dflat = drp.tile([32, 960], I16, tag="dfl")
        nc.sync.dma_start(dflat[:], i16t[:])
        dfv = dflat[:].rearrange("p c -> (p c)")
        # rep[q, k, t*64+z*32+cc] = dflat[(z*16+t)*960 + k*480 + q*30 + cc]
        rep = reps[ci % 2]
        rep4 = rep[:].rearrange("p (k t cc) -> p k t cc", k=2, t=NT, cc=64)
        for k in range(2):
            for z in range(2):
                src = dfv[z * 16 * 960 + k * 480:
                          z * 16 * 960 + k * 480 + 1].copy()
                src.ap = bass_rust.VecI64Pair([[30, 16], [960, NT], [1, 30]])
                nc.scalar.dma_start(rep4[0:16, k, :, z * 32:z * 32 + 30], src)
        # rows 16:127 stay startup-memset zeros: probing whether the HW
        # gather reads only the first 16 idx partitions (CoreSim does)

        # ---- bilinear weight chain -----------------------------------------
        V0 = t64()
        nc.vector.tensor_tensor(V0[:], c0[:], fl[:], A.is_equal)
        V1 = t64()
        nc.vector._custom_dve(OP_EQ_P1, out=V1[:], in0=fl[:], in1=cc1[:])
        # x-side ops stay on base-partition 32 (walrus: TT inputs must share
        # base partition); custom DVE ops mislower at base!=0 on HW, so the
        # x-only F0 keeps the stock 2-op form
        psp = t64()
        nc.vector.tensor_scalar(psp[32:64, :], c0[32:64, :], 1.0, None, A.add)
        F0 = t64()
        nc.vector.tensor_tensor(F0[32:64, :], psp[32:64, :], fl[32:64, :],
                                A.is_equal)
        u = t64()
        nc.vector.tensor_scalar(u[:], frac[:], -1.0, 1.0, A.mult, A.add)
        P0 = t64()
        nc.vector.tensor_tensor(P0[:], u[:], V0[:], A.mult)
        P1 = t64()
        nc.vector.tensor_tensor(P1[:], frac[:], V1[:], A.mult)
        xsA = t64()
        nc.vector.tensor_tensor(xsA[32:64, :], P0[32:64, :], P1[32:64, :], A.add)
        xt = t64()
        nc.vector.tensor_tensor(xt[32:64, :], F0[32:64, :], u[32:64, :], A.mult)
        xt2 = t64()
        nc.vector.tensor_tensor(xt2[32:64, :], V0[32:64, :], frac[32:64, :], A.mult)
        xsB = t64()
        nc.vector.tensor_tensor(xsB[32:64, :], xt[32:64, :], xt2[32:64, :], A.add)
        xs0 = t32()
        nc.vector.tensor_copy(xs0[:], xsA[32:64, :])
        xs1 = t32()
        nc.vector.tensor_copy(xs1[:], xsB[32:64, :])
        A0 = t32()
        nc.vector.tensor_tensor(A0[:], P0[0:32, :], m32[:], A.mult)
        A1 = t32()
        nc.vector.tensor_tensor(A1[:], P1[0:32, :], m32[:], A.mult)

        # wall [32, (slot j, c512)] bf16, j = A(top,pix0) B(top,pix1) C D (bot)
        # 512-col blocks: only cols 0:480 are ever read (pad cols unwritten)
        wall = smi.tile([32, 4 * 512], BF16, tag="wall")
        nc.vector.tensor_tensor(wall[:, 0 * 512:0 * 512 + 480], A0[:], xs0[:], A.mult)
        nc.vector.tensor_tensor(wall[:, 1 * 512:1 * 512 + 480], A0[:], xs1[:], A.mult)
        nc.vector.tensor_tensor(wall[:, 2 * 512:2 * 512 + 480], A1[:], xs0[:], A.mult)
        nc.vector.tensor_tensor(wall[:, 3 * 512:3 * 512 + 480], A1[:], xs1[:], A.mult)
        # t-major wall copy in DRAM: addr = t*3840 + z*1920 + j*480 + c
        wdram = drp.tile([16, 3840], BF16, tag="wdram")
        wflat = wdram[:].rearrange("p f -> (p f)")
        for z in range(2):
            wsr = wall[z * 16:z * 16 + 16, :]
            wsr = wsr.rearrange("p (j c) -> p j c", j=4)[:, :, 0:480]
            wdst = wflat[z * 1920:z * 1920 + 1].copy()
            wdst.ap = bass_rust.VecI64Pair([[3840, NT], [480, 4], [1, 480]])
            nc.sync.dma_start(wdst, wsr)



        # dc accumulator [2][128, (z, 512-block)], 480 used per block
        dcs = [psd.tile([128, 1024], F32, tag=f"dc{oh}", name=f"dc_{ci}_{oh}")
               for oh in range(2)]

        fpv = fp_ap[lvl].copy()
        fpv.ap = bass_rust.VecI64Pair([[C, Hin * Win], [1, 2 * C]])

        RCP = 1024
        for t in range(NT):
            # weight broadcast: wb free (z, j, c480), one DMA per tap
            wb = wbp.tile([128, 4 * RC], BF16, tag="wb")
            src = wflat[t * 3840:t * 3840 + 1].copy()
            src.ap = bass_rust.VecI64Pair([[0, 128], [1, 3840]])
            nc.sync.dma_start(wb[:], src)
            wbj = wb[:].rearrange("p (z j c) -> p j z c", z=2, j=4, c=480)

            gts = []
            for corner in range(2):
                # fp8 gather: 16-bit transpose granularity leaves channel
                # PAIRS per partition; partition cp holds ch (2cp, 2cp+1),
                # free = (pix, i, par). ACT upcasts to bf16 and de-interleaves
                # par to the hl slot: g16 free = (pix, par, i).
                g8 = gat.tile([128, 4 * RCP], FP8, tag=f"g8{corner}")
                g8v = g8[:].rearrange("p (j i) -> p j i", j=4)
                nc.gpsimd.dma_gather(
                    g8v, fpv,
                    rep[:, corner * NT * 64 + t * 64:
                        corner * NT * 64 + (t + 1) * 64],
                    RCP, RCP, 2 * C, elem_step=C,
                    transpose=True, single_packet=False)
                g = g16.tile([128, 4 * RCP], BF16, tag=f"g{corner}")
                g8i = g8[:].rearrange("p (pix i par) -> p pix i par",
                                      pix=2, par=2)
                g16o = g[:].rearrange("p (pix par i) -> p pix i par",
                                      pix=2, par=2)
                # split par-wise: each mult consumes one par half, so its
                # gate is half an upcast; POOL (mostly idle) takes one
                # quarter to debottleneck ACT. Pad samples (c 480:512 per
                # z-block) are skipped — the mults never read them.
                g8z = g8[:].rearrange("p (pix z c par) -> p par pix z c",
                                      pix=2, z=2, c=512)[:, :, :, :, 0:480]
                g16z = g[:].rearrange("p (pix par z c) -> p par pix z c",
                                      pix=2, par=2, z=2)[:, :, :, :, 0:480]
                nc.scalar.activation(g16z[:, 0], g8z[:, 0], AF.Identity)
                if corner == 0:
                    nc.scalar.activation(g16z[:, 1], g8z[:, 1], AF.Identity)
                else:
                    nc.gpsimd.tensor_copy(g16z[:, 1], g8z[:, 1])
                gts.append(g[:].rearrange("p (pix hl z c) -> p hl pix z c",
                                          pix=2, hl=2, z=2))

            ps_ = []
            for corner in range(2):
                for hilo in range(2):
                    p = pp.tile([128, 2 * RC], BF16, tag="p")
                    pv = p[:].rearrange("p (j z c) -> p j z c", j=2, z=2)
                    gsl = gts[corner][:, hilo, :, :, 0:480]     # [128, 2, 2, 480]
                    wsl = wbj[:, 2 * corner:2 * corner + 2]     # [128, 2, 2, 480]
                    nc.vector.tensor_tensor(pv, gsl, wsl, A.mult)
                    ps_.append(p)
            qs = []
            for hilo in range(2):
                q = qp.tile([128, 2 * RC], BF16, tag="q")
                nc.vector.tensor_tensor(q[:], ps_[hilo][:], ps_[2 + hilo][:], A.add)
                qs.append(q)

            for oh in range(2):
                for ih in range(2):
                    for pix in range(2):
                        for z in range(2):
                            nc.tensor.matmul(
                                dcs[oh][:, z * 512:z * 512 + 480],
                                dcn_v[:, t, ih, oh],
                                qs[ih][:, (pix * 2 + z) * 480:
                                        (pix * 2 + z) * 480 + 480],
                                start=(t == 0 and ih == 0 and pix == 0),
                                stop=(t == NT - 1 and ih == 1 and pix == 1))

        # f update: f += relu(dc + b), fused on DVE (per z half: custom DVE
        # ops allow at most 2 free dims)
        for h in range(2):
            fv = fmas[h][:].rearrange("p (r c) -> p r c", c=FW)
            dcv = dcs[h][:].rearrange("p (z c) -> p z c", z=2)[:, :, 0:480]
            dcv = dcv.rearrange("p z (r c) -> p z r c", c=HOUT)
            bias = dcnb_t[:, 2 * lvl + h:2 * lvl + h + 1]
            for z in range(2):
                nc.vector._custom_dve(
                    OP_ADD_RELU,
                    out=fv[:, 1 + z * 12:13 + z * 12, 1:41],
                    in0=fv[:, 1 + z * 12:13 + z * 12, 1:41],
                    in1=dcv[:, z, 0:12, :], s0=bias)
            fsv = fsh[h][:].rearrange("p (r c) -> p r c", c=FW)[:, 1:25, 1:41]
            fiv = fv[:, 1:25, 1:41]
            nc.vector.tensor_copy(fsv, fiv)

    # ---- residual conv + fh ----------------------------------------------
    res_t = cst.tile([128, 9 * 2 * 2 * 128], BF16, tag="dcnw0")
    nc.sync.dma_start(res_t[:], dt["res_w"])
    res_v = res_t[:].rearrange("p (t i o q) -> p t i o q", t=9, i=2, o=2)
    for oh in range(2):
        rps = psd.tile([128, 1024], F32, tag=f"dc{oh}")
        conv3x3(nc, fsh, lambda ti, ih, oh=oh: res_v[:, ti, ih, oh], rps)
        ot = fup.tile([128, RC], F32, tag="ot")
        rpv = rps[:].rearrange("p (z c) -> p z c", z=2)[:, :, 0:480]
        nc.scalar.activation(ot[:], rpv, AF.Identity,
                             bias=resb_t[:, oh:oh + 1])
        nc.vector.tensor_tensor(ot[:], ot[:], fh_t[:, oh * RC:(oh + 1) * RC], A.add)
        nc.sync.dma_start(out_d[128 * oh:128 * (oh + 1), :], ot[:])


def conv3x3(nc, fsh, w_fn, out_ps):
    """3x3 stride-1 conv over the padded f window; out [cout, 960]."""
    taps = [(a, b) for a in (-1, 0, 1) for b in (-1, 0, 1)]
    for ti, (dy, dx) in enumerate(taps):
        for ih in range(2):
            rhs = fsh[ih][:].rearrange("p (r c) -> p r c", c=FW)
            for nh in range(2):
                nc.tensor.matmul(
                    out_ps[:, nh * 512:nh * 512 + 480],
                    w_fn(ti, ih),
                    rhs[:, 1 + dy + nh * 12:1 + dy + nh * 12 + 12,
                        1 + dx:1 + dx + 40],
                    start=(ti == 0 and ih == 0), stop=(ti == 8 and ih == 1))


# ===========================================================================
# host side
# ===========================================================================

def prep_core_inputs(inputs, b, half):
    """Per-core input map for image b, row-half `half` (0=top)."""
    g0 = 0 if half == 0 else 16
    f0 = np.asarray(inputs["f0"][b], np.float32)
    f1 = np.asarray(inputs["f1"][b], np.float32)
    f2 = np.asarray(inputs["f2"][b], np.float32)

    def pix_table(f):
        hw = f.shape[1] * f.shape[2]
        t = np.zeros((hw + 1, C), np.float32)
        t[:hw] = f.transpose(1, 2, 0).reshape(hw, C)
        return t.astype(ml_dtypes.float8_e4m3fn)

    finit = np.zeros((C, FR, FW), np.float32)
    for r in range(FR):
        gr = g0 - 1 + r
        if 0 <= gr < HOUT:
            finit[:, r, 1:41] = f2[:, gr, :]

    # fh as [128, (oh, rc)]
    fh0 = f2[:, g0:g0 + ROWS, :].reshape(C, RC)
    fh = np.concatenate([fh0[:128], fh0[128:]], axis=1)

    byx = np.zeros((2, 64, 480), np.float32)
    hi0 = np.zeros((2, 64, 1), np.float32)
    for lvl in range(2):
        k_, st_, pad_, dil_ = CONFIGS[lvl]
        Hin = HIN[lvl]
        rc = np.arange(480)
        for rcb in range(2):
            rr = (rcb * 480 + rc) // HOUT
            cc = (rcb * 480 + rc) % HOUT
            for t in range(NT):
                byx[lvl, rcb * 16 + t] = st_ * (g0 + rr) - pad_ + (t // k_) * dil_
                byx[lvl, 32 + rcb * 16 + t] = st_ * cc - pad_ + (t % k_) * dil_
        hi0[lvl, 0:32] = Hin - 1
        hi0[lvl, 32:64] = Hin - 2
    byx = byx.transpose(1, 0, 2).reshape(64, 2 * 480)
    hi0 = hi0.transpose(1, 0, 2).reshape(64, 2)

    perm = list(range(0, 32, 2)) + list(range(1, 32, 2)) + list(range(32, 48))
    com_w = np.zeros((2, 9, 2, 128, 48), np.float32)
    com_b = np.zeros((2, 48, 1), np.float32)
    dcn_w = np.zeros((2, NT, 2, 2, 128, 128), np.float32)
    dcn_b = np.zeros((2, 2, 128, 1), np.float32)
    for lvl in range(2):
        cw = np.asarray(inputs[f"com_w{lvl}"], np.float32)[perm]
        cb = np.asarray(inputs[f"com_b{lvl}"], np.float32)[perm]
        for ty in range(3):
            for tx in range(3):
                for ih in range(2):
                    com_w[lvl, ty * 3 + tx, ih] = \
                        cw[:, ih * 128:(ih + 1) * 128, ty, tx].T
        com_b[lvl, :, 0] = cb
        dw = np.asarray(inputs[f"dcn_w{lvl}"], np.float32)
        # fp8 gather leaves ch pairs per partition: row cp of "ih" slot par
        # is input channel 2*cp + par
        for k in range(NT):
            for par in range(2):
                for oh in range(2):
                    dcn_w[lvl, k, par, oh] = dw[oh * 128:(oh + 1) * 128,
                                                par::2,
                                                k // 4, k % 4].T
        db = np.asarray(inputs[f"dcn_b{lvl}"], np.float32)
        dcn_b[lvl, 0, :, 0] = db[:128]
        dcn_b[lvl, 1, :, 0] = db[128:]
    rw = np.asarray(inputs["res_w"], np.float32)
    res_w = np.zeros((9, 2, 2, 128, 128), np.float32)
    for ty in range(3):
        for tx in range(3):
            for ih in range(2):
                for oh in range(2):
                    res_w[ty * 3 + tx, ih, oh] = rw[oh * 128:(oh + 1) * 128,
                                                    ih * 128:(ih + 1) * 128,
                                                    ty, tx].T
    rb = np.asarray(inputs["res_b"], np.float32)
    res_b = np.stack([rb[:128], rb[128:]], axis=1)  # [128, 2]

    # transpose weight stacks to [partition, ...] DRAM layouts
    com_w = com_w.transpose(3, 0, 1, 2, 4).reshape(128, -1)
    com_b = com_b.transpose(1, 0, 2).reshape(48, 2)
    dcn_w = dcn_w.transpose(0, 4, 1, 2, 3, 5).reshape(2, 128, -1)
    dcn_b = dcn_b.transpose(2, 0, 1, 3).reshape(128, 4)
    res_w = res_w.transpose(3, 0, 1, 2, 4).reshape(128, -1)

    return {
        "fp0": pix_table(f0),
        "fp1": pix_table(f1),
        "finit": finit.reshape(C, FSZ),
        "fh": fh.astype(ml_dtypes.bfloat16),
        "byx": byx,
        "hi0": hi0,
        "com_w": com_w.astype(ml_dtypes.bfloat16),
        "com_b": np.ascontiguousarray(com_b),
        "dcn_w": np.ascontiguousarray(dcn_w).astype(ml_dtypes.bfloat16),
        "dcn_b": np.ascontiguousarray(dcn_b),
        "res_w": np.ascontiguousarray(res_w).astype(ml_dtypes.bfloat16),
        "res_b": np.ascontiguousarray(res_b).astype(np.float32),
    }


def assemble_output(results):
    out = np.zeros((B, C, HOUT, HOUT), np.float32)
    for b in range(B):
        top = np.asarray(results[2 * b]["out"]).reshape(C, ROWS, HOUT)
        bot = np.asarray(results[2 * b + 1]["out"]).reshape(C, ROWS, HOUT)
        out[b, :, 0:20, :] = top[:, 0:20, :]
        out[b, :, 20:40, :] = bot[:, 4:24, :]
    return out


_NC_CACHE = []


def kernel(**inputs):
    if not _NC_CACHE:
        _NC_CACHE.append(build_program())
    nc = _NC_CACHE[0]
    in_maps = [prep_core_inputs(inputs, b, half)
               for b in range(B) for half in range(2)]
    from concourse.bass_utils import run_bass_kernel_spmd
    r = run_bass_kernel_spmd(nc, in_maps, list(range(8)))
    return assemble_output(r.results)

